# revision 35
# baseline (speedup 1.0000x reference)
"""Single-head causal attention (B=4, T=4096, E=1024, H=128) on 8 trn2 cores.

Sharding: core c -> (batch b = c//2, piece p = c%2). Within a batch the 32
query blocks of 128 rows are split even/odd between the two pieces so the
causal workload balances. The device program is identical on all cores
(SPMD); all per-core differences are carried by the input data:
  - xT arrives column-PERMUTED per core: within every 256-token block the
    core's own 128 query tokens come first. The Q projection then reads a
    fixed strided slice of the same x tiles used for K/V (no separate
    gathered copy), and the program stays core-independent.
  - the causal-boundary mask strip is per-core data.

Device algorithm (per core, all "transposed" layouts):
  per round tt (1024 permuted tokens = key blocks 8tt..8tt+7):
    KT chunk = Wk @ x^T            [H=128, 512] x2
    QT tile  = Wq @ xq^T           [H=128, 512]   (strided in-tile gather)
    VT chunk = Wv @ x^T -> f16 -> PE-transpose -> V blocks [128 tok, 128 h]
    attention for q-tile tt over kb in 0..8tt+7:
      ST[kb] = KT_blk^T @ QT_tile  [128 k, 512 q] (PSUM, c0-trimmed)
      boundary block gets an additive causal mask strip (per-core data)
      PT = exp(scale*ST) f16; OT += V_blk^T @ PT; pacc[kb%2] += PT (DVE)
      lb = allones^T @ (pacc0+pacc1)   [128, 512] broadcast denominator
      O = OT * reciprocal_approx(lb) stored as out[:, tile] in [H, TQ]
  projections are software-pipelined: Q/K-A/V-A of round tt+1 interleave
  into attention round tt; K-B/V-B may spill into attention round tt+1
  itself (their key blocks are only read from iteration 8(tt+1)+4).
Host transposes [H, TQ] -> [TQ, H] when scattering into the full output.
"""

import numpy as np

B, T, E, H = 4, 4096, 1024, 128
P = 128
NB_E = E // P           # 8 contraction chunks
TQ = T // 2             # 2048 gathered queries per core
N_RND = 4               # rounds; round tt = permuted tokens 1024tt..+1023
SCALE = float(H) ** -0.5
NEG = -30000.0
N_CORES = 8
F32 = np.float32


def _query_rows(p: int) -> np.ndarray:
    """Absolute row indices of the gathered queries for piece p (in order)."""
    blocks = [np.arange(256 * g + 128 * p, 256 * g + 128 * p + 128) for g in range(16)]
    return np.concatenate(blocks)


def _perm_cols(p: int) -> np.ndarray:
    """Permuted token order for core piece p: within each 256-token block the
    own 128 tokens (offset 128p) come first, the other 128 after."""
    out = []
    for g in range(16):
        own = np.arange(256 * g + 128 * p, 256 * g + 128 * p + 128)
        oth = np.arange(256 * g + 128 * (1 - p), 256 * g + 128 * (1 - p) + 128)
        out.append(own)
        out.append(oth)
    return np.concatenate(out)


def _mask_strip(p: int) -> np.ndarray:
    """maskT [1024 k, 512 q] f16: 0 where key visible, NEG where masked.

    Row 128*j + kk is PERMUTED in-strip key block j (j=0..7); col 128*i + r
    is in-tile query block i. Permuted block j holds original block
    jp = j + p*(1-2*(j%2)); visible iff 128*jp + kk <= 256*i + 128*p + r.
    """
    j = np.arange(1024)[:, None] // 128
    kk = np.arange(1024)[:, None] % 128
    jp = j + p * (1 - 2 * (j % 2))
    qq = np.arange(512)[None, :]
    i, r = qq // 128, qq % 128
    visible = 128 * jp + kk <= 256 * i + 128 * p + r
    return np.where(visible, 0.0, NEG).astype(np.float16)


def _c0_of(tt: int, kb: int) -> int:
    """First un-skippable query column for key block kb in q-tile tt."""
    if kb < 8 * tt:
        return 0
    j = kb - 8 * tt
    return P * max(0, -(-(128 * j - 255) // 256))


def _emit(tc, aps):
    from concourse import mybir

    nc = tc.nc
    f32 = mybir.dt.float32
    f16 = mybir.dt.float16
    f8 = mybir.dt.float8e4
    DR = mybir.MatmulPerfMode.DoubleRow
    EXP = mybir.ActivationFunctionType.Exp

    from concourse.masks import make_identity
    from contextlib import ExitStack

    xT, wqkv, maskT, out = aps

    ctx = ExitStack()
    with ctx:
        # ---- pools ----
        consts = ctx.enter_context(tc.tile_pool(name="consts", bufs=1))
        x_pool = ctx.enter_context(tc.tile_pool(name="x", bufs=3))
        qt_pool = ctx.enter_context(tc.tile_pool(name="qt", bufs=2))
        vt_pool = ctx.enter_context(tc.tile_pool(name="vt", bufs=2))
        pt_pool = ctx.enter_context(tc.tile_pool(name="pt", bufs=4))
        pa_pool = ctx.enter_context(tc.tile_pool(name="pa", bufs=4))
        osb_pool = ctx.enter_context(tc.tile_pool(name="osb", bufs=2))
        rl_pool = ctx.enter_context(tc.tile_pool(name="rl", bufs=2))
        s_ps = ctx.enter_context(tc.tile_pool(name="sps", bufs=3, space="PSUM"))
        o_ps = ctx.enter_context(tc.tile_pool(name="ops", bufs=2, space="PSUM"))
        p_ps = ctx.enter_context(tc.tile_pool(name="pps", bufs=2, space="PSUM"))
        lb_ps = ctx.enter_context(tc.tile_pool(name="lps", bufs=1, space="PSUM"))

        # ---- persistent SBUF tensors ----
        identity = consts.tile([P, P], f16)
        allones = consts.tile([P, P], f16)
        w_sb = consts.tile([P, NB_E, 3, P], f16)   # [., chunk, (k|v|q), .]
        mask_sb = consts.tile([P, 8, 512], f16)
        kt_all = consts.tile([P, T // P, P], f16)
        v_all = consts.tile([P, T // P, P], f16)

        make_identity(nc, identity[:])
        nc.gpsimd.memset(allones[:], 1.0)

        # ---- x round tiles: [128, chunk c, block b, 256] f16 ----
        x_tiles = [x_pool.tile([P, NB_E, 4, 256], f16, tag="x", name=f"x_{tt}")
                   for tt in range(N_RND)]

        def dma_x(tt, halves, split=1):
            """Load round tt's 1024 permuted tokens (halves: list of 0/1).
            split>1 slices the load along E-chunks for finer semaphores."""
            for h in halves:
                t0 = tt * 1024 + h * 512
                cw = NB_E // split
                for g in range(split):
                    nc.sync.dma_start(
                        x_tiles[tt][:, g * cw:(g + 1) * cw, 2 * h:2 * h + 2, :],
                        xT[g * cw * P:(g + 1) * cw * P, t0:t0 + 512].rearrange(
                            "(c p) (b q) -> p c b q", p=P, b=2),
                    )

        # startup-critical order: weights, then round-0 x at chunk-pair
        # granularity (the first K matmuls gate only on their own chunks),
        # mask strips after (first needed mid-attention-0), bulk x last.
        for ch in (0, 4):
            nc.sync.dma_start(
                w_sb[:, ch:ch + 4, :, :],
                wqkv[ch * P:(ch + 4) * P, :, :].rearrange(
                    "(c p) s h -> p c s h", p=P))
        dma_x(0, [0], split=4)
        dma_x(0, [1], split=2)
        for jh in (0, 1):
            nc.sync.dma_start(
                mask_sb[:, 4 * jh:4 * jh + 4, :],
                maskT[512 * jh:512 * jh + 512, :].rearrange(
                    "(j p) q -> p j q", p=P))
        dma_x(1, [0, 1])
        dma_x(2, [0, 1])

        # PE warmup: dummy transposes ramp the tensor-engine p-state while
        # the first DMAs stream; their results are never read.
        for _ in range(28):
            wp = p_ps.tile([P, P], f16, tag="pps", name="warm")
            nc.tensor.transpose(wp[:], identity[:], identity[:])

        # ---- projection pieces (generators of thunks) ----
        def mk_group(tt, sel, rhs_fn, fin):
            ps = p_ps.tile([P, 512], f32, tag="pps")
            for c in range(NB_E):
                def mm(c=c, ps=ps):
                    nc.tensor.matmul(ps[:], lhsT=w_sb[:, c, sel, :],
                                     rhs=rhs_fn(c), start=(c == 0),
                                     stop=(c == NB_E - 1))
                yield mm
            yield lambda ps=ps: fin(ps)

        def v_group(tt, h):
            xt = x_tiles[tt]
            vt = vt_pool.tile([P, 512], f16, tag="vt", name=f"vt_{tt}_{h}")

            def fin_v(ps, vt=vt):
                nc.scalar.copy(vt[:], ps[:])
            yield from mk_group(
                tt, 1, lambda c, h=h: xt[:, c, 2 * h:2 * h + 2, :], fin_v)
            for u in range(4):
                kb = tt * 8 + 4 * h + u

                def tr(u=u, kb=kb, vt=vt):
                    tp = p_ps.tile([P, P], f16, tag="pps",
                                   name=f"tp_{tt}_{kb}")
                    nc.tensor.transpose(tp[:], vt[:, u * P:(u + 1) * P],
                                        identity[:])
                    nc.vector.tensor_copy(v_all[:, kb, :], tp[:])
                yield tr

        def k_group(tt, h):
            xt = x_tiles[tt]

            def fin_k(ps):
                dst = kt_all[:, tt * 8 + 4 * h: tt * 8 + 4 * h + 4, :]
                nc.scalar.copy(dst, ps[:])
            yield from mk_group(
                tt, 0, lambda c: xt[:, c, 2 * h:2 * h + 2, :], fin_k)

        def q_group(tt, qt):
            xt = x_tiles[tt]

            def fin_q(ps):
                nc.scalar.copy(qt[:], ps[:])
            yield from mk_group(tt, 2, lambda c: xt[:, c, :, 0:128], fin_q)

        def chain(*gens):
            for g in gens:
                yield from g

        def drain(gen, n):
            """Emit up to n pieces; returns False when exhausted."""
            if gen is None:
                return False
            for _ in range(n):
                try:
                    next(gen)()
                except StopIteration:
                    return False
            return True

        qts = [qt_pool.tile([P, 512], f16, tag="qt", name=f"qt_{t}")
               for t in range(N_RND)]

        # round 0 projections run up front (halves as their DMAs land)
        for piece in chain(k_group(0, 0), v_group(0, 0), k_group(0, 1),
                           q_group(0, qts[0])):
            piece()

        # interleave/spill generators:
        #   head(r) = Q + K-A + V-A of round r  (into attention r-1)
        #   tail(r) = K-B + V-B of round r      (may spill into attention r)
        head = lambda r: q_group(r, qts[r])
        tail = lambda r: chain(k_group(r, 0), v_group(r, 0),
                               k_group(r, 1), v_group(r, 1))

        gen_head = None   # projections of round tt+1, must finish in attn tt
        gen_tail = None   # projections of round tt+1, may spill into attn tt+1
        spill = v_group(0, 1)  # leftovers with deadline in the current round

        for tt in range(N_RND):
            if tt == 1:
                dma_x(3, [0, 1])
            if tt + 1 < N_RND:
                gen_head = chain(head(tt + 1))
                gen_tail = chain(tail(tt + 1))
            else:
                gen_head = gen_tail = None

            nkb = 8 * tt + 8
            # pacing: head must drain within this round's attention;
            # tail may run behind and spill (deadline in next round).
            n_head = 9  # Q group only; K/V spill into their own round
            per_head = -(-n_head // max(1, nkb - 2))

            qs = qts[tt]
            ot = o_ps.tile([P, 512], f32, tag="ops", name=f"ot_{tt}")
            lb = lb_ps.tile([P, 512], f32, tag="lps", name=f"lb_{tt}")
            pacc = [pa_pool.tile([P, 512], f16, tag="pa", name=f"pa_{tt}_{i}")
                    for i in range(2)]
            for i in range(2):
                nc.gpsimd.memset(pacc[i][:], 0.0)

            s_tiles = [None] * nkb

            def emit_score(kb, tt=tt, qs=qs, s_tiles=s_tiles):
                c0 = _c0_of(tt, kb)
                s = s_ps.tile([P, 512], f32, tag="sps", name=f"s_{tt}_{kb}")
                nc.tensor.matmul(
                    s[:, c0:512],
                    lhsT=kt_all[:, kb, :],
                    rhs=qs[:, c0:512],
                    start=True, stop=True,
                )
                s_tiles[kb] = s

            emit_score(0)
            if nkb > 1:
                emit_score(1)
            for kb in range(nkb):
                # spilled tail of THIS round's projections: its key blocks
                # (8tt+4..8tt+7) are first read at iteration 8tt+3 (score
                # lookahead); keep a safe margin.
                sp_dl, sp_rate = (4, 4) if tt == 0 else (8 * tt + 2, 4)
                if spill is not None and kb < sp_dl:
                    if not drain(spill, sp_rate):
                        spill = None

                if kb + 2 < nkb:
                    emit_score(kb + 2)
                s = s_tiles[kb]
                c0 = _c0_of(tt, kb)
                if kb >= 8 * tt:
                    j = kb - 8 * tt
                    nc.vector.tensor_add(
                        s[:, c0:c0 + P], s[:, c0:c0 + P],
                        mask_sb[:, j, c0:c0 + P])
                pt = pt_pool.tile([P, 512], f16, tag="pt")
                nc.scalar.activation(pt[:, c0:512], s[:, c0:512], EXP,
                                     scale=SCALE)
                nc.tensor.matmul(
                    ot[:, c0:512],
                    lhsT=v_all[:, kb, :],
                    rhs=pt[:, c0:512],
                    start=(kb == 0),
                    stop=(kb == nkb - 1),
                )
                nc.vector.tensor_add(pacc[kb % 2][:, c0:512],
                                     pacc[kb % 2][:, c0:512],
                                     pt[:, c0:512])

                if gen_head is not None:
                    if not drain(gen_head, per_head):
                        gen_head = None
                elif gen_tail is not None and kb >= nkb - 8:
                    if not drain(gen_tail, 3):
                        gen_tail = None

            # anything left of the head must land now; tail becomes spill
            while drain(gen_head, 4):
                pass
            gen_head = None
            spill = gen_tail
            gen_tail = None

            # epilogue: softmax denominator + normalize + store (the last
            # tile streams in halves so the store overlaps the arithmetic)
            halves = (0, 256) if tt == N_RND - 1 else (0,)
            width = 512 // len(halves)
            for hb in halves:
                sl = slice(hb, hb + width)
                nc.vector.tensor_add(pacc[0][:, sl], pacc[0][:, sl],
                                     pacc[1][:, sl])
                nc.tensor.matmul(lb[:, sl], lhsT=allones[:],
                                 rhs=pacc[0][:, sl], start=True, stop=True)
                rl = rl_pool.tile([P, width], f32, tag="rl")
                nc.vector.reciprocal_approx_fast(rl[:], lb[:, sl])
                o_sb = osb_pool.tile([P, width], f32, tag="osb")
                nc.vector.tensor_mul(o_sb[:], ot[:, sl], rl[:])
                nc.gpsimd.dma_start(
                    out[:, tt * 512 + hb: tt * 512 + hb + width], o_sb[:])


def build_program():
    import concourse.tile as tile
    from concourse import bacc, mybir

    f32 = mybir.dt.float32
    f16 = mybir.dt.float16
    nc = bacc.Bacc("TRN2", target_bir_lowering=False, debug=False,
                   num_devices=N_CORES)
    xT = nc.dram_tensor("xT", [E, T], f16, kind="ExternalInput").ap()
    wqkv = nc.dram_tensor("wqkv", [E, 3, H], f16, kind="ExternalInput").ap()
    maskT = nc.dram_tensor("maskT", [1024, 512], f16, kind="ExternalInput").ap()
    out = nc.dram_tensor("out", [H, TQ], f32, kind="ExternalOutput").ap()

    with tile.TileContext(nc) as tc:
        _emit(tc, (xT, wqkv, maskT, out))
    nc.compile()
    return nc


def make_in_maps(x, Wq, Wk, Wv):
    """Per-core input maps. x: [B,T,E] f32; W*: [H,E] f32."""
    x = np.asarray(x, dtype=F32)
    # combined [E, 3, H] with slot order (k, v, q)
    wqkv = np.stack(
        [np.asarray(Wk, F32).T, np.asarray(Wv, F32).T, np.asarray(Wq, F32).T],
        axis=1).astype(np.float16)
    wqkv = np.ascontiguousarray(wqkv)
    masks = [_mask_strip(0), _mask_strip(1)]
    perms = [_perm_cols(0), _perm_cols(1)]
    in_maps = []
    for c in range(N_CORES):
        b, p = c // 2, c % 2
        xb = x[b][perms[p]]                                    # [T, E] permuted
        xT_np = np.ascontiguousarray(xb.T.astype(np.float16))
        in_maps.append({
            "xT": xT_np,
            "wqkv": wqkv,
            "maskT": masks[p],
        })
    return in_maps


def run(x, Wq, Wk, Wv, trace=False, trace_cores=None):
    """Returns (full_output [B,T,H] f32, BassKernelResults)."""
    from concourse.bass_utils import run_bass_kernel_spmd

    nc = build_program()
    in_maps = make_in_maps(x, Wq, Wk, Wv)
    res = run_bass_kernel_spmd(
        nc, in_maps, list(range(N_CORES)), trace=trace,
        trace_cores=trace_cores,
    )
    full = np.empty((B, T, H), dtype=F32)
    for c in range(N_CORES):
        b, p = c // 2, c % 2
        full[b, _query_rows(p), :] = res.results[c]["out"].T
    return full, res


def kernel(x, Wq, Wk, Wv):
    full, _ = run(x, Wq, Wk, Wv, trace=False)
    return full


if __name__ == "__main__":
    nc = build_program()
    print("program built ok")


# revision 36
# speedup vs baseline: 1.0061x; 1.0061x over previous
"""Single-head causal attention (B=4, T=4096, E=1024, H=128) on 8 trn2 cores.

Sharding: core c -> (batch b = c//2, piece p = c%2). Within a batch the 32
query blocks of 128 rows are split even/odd between the two pieces so the
causal workload balances. The device program is identical on all cores
(SPMD); all per-core differences are carried by the input data:
  - xT arrives column-PERMUTED per core: within every 256-token block the
    core's own 128 query tokens come first. The Q projection then reads a
    fixed strided slice of the same x tiles used for K/V (no separate
    gathered copy), and the program stays core-independent.
  - the causal-boundary mask strip is per-core data.

Device algorithm (per core, all "transposed" layouts):
  per round tt (1024 permuted tokens = key blocks 8tt..8tt+7):
    KT chunk = Wk @ x^T            [H=128, 512] x2
    QT tile  = Wq @ xq^T           [H=128, 512]   (strided in-tile gather)
    VT chunk = Wv @ x^T -> f16 -> PE-transpose -> V blocks [128 tok, 128 h]
    attention for q-tile tt over kb in 0..8tt+7:
      ST[kb] = KT_blk^T @ QT_tile  [128 k, 512 q] (PSUM, c0-trimmed)
      boundary block gets an additive causal mask strip (per-core data)
      PT = exp(scale*ST) f16; OT += V_blk^T @ PT; pacc[kb%2] += PT (DVE)
      lb = allones^T @ (pacc0+pacc1)   [128, 512] broadcast denominator
      O = OT * reciprocal_approx(lb) stored as out[:, tile] in [H, TQ]
  projections are software-pipelined: Q/K-A/V-A of round tt+1 interleave
  into attention round tt; K-B/V-B may spill into attention round tt+1
  itself (their key blocks are only read from iteration 8(tt+1)+4).
Host transposes [H, TQ] -> [TQ, H] when scattering into the full output.
"""

import numpy as np

B, T, E, H = 4, 4096, 1024, 128
P = 128
NB_E = E // P           # 8 contraction chunks
TQ = T // 2             # 2048 gathered queries per core
N_RND = 4               # rounds; round tt = permuted tokens 1024tt..+1023
SCALE = float(H) ** -0.5
NEG = -30000.0
N_CORES = 8
F32 = np.float32


def _query_rows(p: int) -> np.ndarray:
    """Absolute row indices of the gathered queries for piece p (in order)."""
    blocks = [np.arange(256 * g + 128 * p, 256 * g + 128 * p + 128) for g in range(16)]
    return np.concatenate(blocks)


def _perm_cols(p: int) -> np.ndarray:
    """Permuted token order for core piece p: within each 256-token block the
    own 128 tokens (offset 128p) come first, the other 128 after."""
    out = []
    for g in range(16):
        own = np.arange(256 * g + 128 * p, 256 * g + 128 * p + 128)
        oth = np.arange(256 * g + 128 * (1 - p), 256 * g + 128 * (1 - p) + 128)
        out.append(own)
        out.append(oth)
    return np.concatenate(out)


def _mask_strip(p: int) -> np.ndarray:
    """maskT [1024 k, 512 q] f16: 0 where key visible, NEG where masked.

    Row 128*j + kk is PERMUTED in-strip key block j (j=0..7); col 128*i + r
    is in-tile query block i. Permuted block j holds original block
    jp = j + p*(1-2*(j%2)); visible iff 128*jp + kk <= 256*i + 128*p + r.
    """
    j = np.arange(1024)[:, None] // 128
    kk = np.arange(1024)[:, None] % 128
    jp = j + p * (1 - 2 * (j % 2))
    qq = np.arange(512)[None, :]
    i, r = qq // 128, qq % 128
    visible = 128 * jp + kk <= 256 * i + 128 * p + r
    return np.where(visible, 0.0, NEG).astype(np.float16)


def _c0_of(tt: int, kb: int) -> int:
    """First un-skippable query column for key block kb in q-tile tt."""
    if kb < 8 * tt:
        return 0
    j = kb - 8 * tt
    return P * max(0, -(-(128 * j - 255) // 256))


def _emit(tc, aps):
    from concourse import mybir

    nc = tc.nc
    f32 = mybir.dt.float32
    f16 = mybir.dt.float16
    f8 = mybir.dt.float8e4
    DR = mybir.MatmulPerfMode.DoubleRow
    EXP = mybir.ActivationFunctionType.Exp

    from concourse.masks import make_identity
    from contextlib import ExitStack

    xT, wqkv, maskT, out = aps

    ctx = ExitStack()
    with ctx:
        # ---- pools ----
        consts = ctx.enter_context(tc.tile_pool(name="consts", bufs=1))
        x_pool = ctx.enter_context(tc.tile_pool(name="x", bufs=3))
        qt_pool = ctx.enter_context(tc.tile_pool(name="qt", bufs=2))
        vt_pool = ctx.enter_context(tc.tile_pool(name="vt", bufs=2))
        pt_pool = ctx.enter_context(tc.tile_pool(name="pt", bufs=4))
        pa_pool = ctx.enter_context(tc.tile_pool(name="pa", bufs=4))
        osb_pool = ctx.enter_context(tc.tile_pool(name="osb", bufs=2))
        rl_pool = ctx.enter_context(tc.tile_pool(name="rl", bufs=2))
        s_ps = ctx.enter_context(tc.tile_pool(name="sps", bufs=3, space="PSUM"))
        o_ps = ctx.enter_context(tc.tile_pool(name="ops", bufs=2, space="PSUM"))
        p_ps = ctx.enter_context(tc.tile_pool(name="pps", bufs=2, space="PSUM"))
        lb_ps = ctx.enter_context(tc.tile_pool(name="lps", bufs=1, space="PSUM"))

        # ---- persistent SBUF tensors ----
        identity = consts.tile([P, P], f16)
        allones = consts.tile([P, P], f16)
        w_sb = consts.tile([P, NB_E, 3, P], f16)   # [., chunk, (k|v|q), .]
        mask_sb = consts.tile([P, 8, 512], f16)
        kt_all = consts.tile([P, T // P, P], f16)
        v_all = consts.tile([P, T // P, P], f16)

        make_identity(nc, identity[:])
        nc.gpsimd.memset(allones[:], 1.0)

        # ---- x round tiles: [128, chunk c, block b, 256] f16 ----
        x_tiles = [x_pool.tile([P, NB_E, 4, 256], f16, tag="x", name=f"x_{tt}")
                   for tt in range(N_RND)]

        def dma_x(tt, halves, split=1):
            """Load round tt's 1024 permuted tokens (halves: list of 0/1).
            split>1 slices the load along E-chunks for finer semaphores."""
            for h in halves:
                t0 = tt * 1024 + h * 512
                cw = NB_E // split
                for g in range(split):
                    nc.sync.dma_start(
                        x_tiles[tt][:, g * cw:(g + 1) * cw, 2 * h:2 * h + 2, :],
                        xT[g * cw * P:(g + 1) * cw * P, t0:t0 + 512].rearrange(
                            "(c p) (b q) -> p c b q", p=P, b=2),
                    )

        # startup-critical order: weights, then round-0 x at chunk-pair
        # granularity (the first K matmuls gate only on their own chunks),
        # mask strips after (first needed mid-attention-0), bulk x last.
        for ch in (0, 4):
            nc.sync.dma_start(
                w_sb[:, ch:ch + 4, :, :],
                wqkv[ch * P:(ch + 4) * P, :, :].rearrange(
                    "(c p) s h -> p c s h", p=P))
        dma_x(0, [0], split=4)
        dma_x(0, [1], split=2)
        for jh in (0, 1):
            nc.sync.dma_start(
                mask_sb[:, 4 * jh:4 * jh + 4, :],
                maskT[512 * jh:512 * jh + 512, :].rearrange(
                    "(j p) q -> p j q", p=P))
        dma_x(1, [0, 1])
        dma_x(2, [0, 1])

        # PE warmup: dummy transposes ramp the tensor-engine p-state while
        # the first DMAs stream; their results are never read.
        for _ in range(28):
            wp = p_ps.tile([P, P], f16, tag="pps", name="warm")
            nc.tensor.transpose(wp[:], identity[:], identity[:])

        # ---- projection pieces (generators of thunks) ----
        def mk_group(tt, sel, rhs_fn, fin):
            ps = p_ps.tile([P, 512], f32, tag="pps")
            for c in range(NB_E):
                def mm(c=c, ps=ps):
                    nc.tensor.matmul(ps[:], lhsT=w_sb[:, c, sel, :],
                                     rhs=rhs_fn(c), start=(c == 0),
                                     stop=(c == NB_E - 1))
                yield mm
            yield lambda ps=ps: fin(ps)

        def v_group(tt, h):
            xt = x_tiles[tt]
            vt = vt_pool.tile([P, 512], f16, tag="vt", name=f"vt_{tt}_{h}")

            def fin_v(ps, vt=vt):
                nc.scalar.copy(vt[:], ps[:])
            yield from mk_group(
                tt, 1, lambda c, h=h: xt[:, c, 2 * h:2 * h + 2, :], fin_v)
            for u in range(4):
                kb = tt * 8 + 4 * h + u

                def tr(u=u, kb=kb, vt=vt):
                    tp = p_ps.tile([P, P], f16, tag="pps",
                                   name=f"tp_{tt}_{kb}")
                    nc.tensor.transpose(tp[:], vt[:, u * P:(u + 1) * P],
                                        identity[:])
                    nc.vector.tensor_copy(v_all[:, kb, :], tp[:])
                yield tr

        def k_group(tt, h):
            xt = x_tiles[tt]

            def fin_k(ps):
                dst = kt_all[:, tt * 8 + 4 * h: tt * 8 + 4 * h + 4, :]
                nc.scalar.copy(dst, ps[:])
            yield from mk_group(
                tt, 0, lambda c: xt[:, c, 2 * h:2 * h + 2, :], fin_k)

        def q_group(tt, qt):
            xt = x_tiles[tt]

            def fin_q(ps):
                nc.scalar.copy(qt[:], ps[:])
            yield from mk_group(tt, 2, lambda c: xt[:, c, :, 0:128], fin_q)

        def chain(*gens):
            for g in gens:
                yield from g

        def drain(gen, n):
            """Emit up to n pieces; returns False when exhausted."""
            if gen is None:
                return False
            for _ in range(n):
                try:
                    next(gen)()
                except StopIteration:
                    return False
            return True

        qts = [qt_pool.tile([P, 512], f16, tag="qt", name=f"qt_{t}")
               for t in range(N_RND)]

        # round 0 projections run up front (halves as their DMAs land)
        for piece in chain(k_group(0, 0), v_group(0, 0), k_group(0, 1),
                           q_group(0, qts[0])):
            piece()

        # interleave/spill generators:
        #   head(r) = Q + K-A + V-A of round r  (into attention r-1)
        #   tail(r) = K-B + V-B of round r      (may spill into attention r)
        head = lambda r: chain(q_group(r, qts[r]), k_group(r, 0),
                               v_group(r, 0))
        tail = lambda r: chain(k_group(r, 1), v_group(r, 1))

        gen_head = None   # projections of round tt+1, must finish in attn tt
        gen_tail = None   # projections of round tt+1, may spill into attn tt+1
        spill = v_group(0, 1)  # leftovers with deadline in the current round

        for tt in range(N_RND):
            if tt == 1:
                dma_x(3, [0, 1])
            if tt + 1 < N_RND:
                gen_head = chain(head(tt + 1))
                gen_tail = chain(tail(tt + 1))
            else:
                gen_head = gen_tail = None

            nkb = 8 * tt + 8
            # pacing: head must drain within this round's attention;
            # tail may run behind and spill (deadline in next round).
            n_head = 31  # 3 groups x 9 + 4 transposes
            per_head = -(-n_head // max(1, nkb - 2))

            qs = qts[tt]
            ot = o_ps.tile([P, 512], f32, tag="ops", name=f"ot_{tt}")
            lb = lb_ps.tile([P, 512], f32, tag="lps", name=f"lb_{tt}")
            pacc = [pa_pool.tile([P, 512], f16, tag="pa", name=f"pa_{tt}_{i}")
                    for i in range(2)]
            for i in range(2):
                nc.gpsimd.memset(pacc[i][:], 0.0)

            s_tiles = [None] * nkb

            def emit_score(kb, tt=tt, qs=qs, s_tiles=s_tiles):
                c0 = _c0_of(tt, kb)
                s = s_ps.tile([P, 512], f32, tag="sps", name=f"s_{tt}_{kb}")
                nc.tensor.matmul(
                    s[:, c0:512],
                    lhsT=kt_all[:, kb, :],
                    rhs=qs[:, c0:512],
                    start=True, stop=True,
                )
                s_tiles[kb] = s

            emit_score(0)
            if nkb > 1:
                emit_score(1)
            for kb in range(nkb):
                # spilled tail of THIS round's projections: its key blocks
                # (8tt+4..8tt+7) are first read at iteration 8tt+3 (score
                # lookahead); keep a safe margin.
                sp_dl, sp_rate = (4, 4) if tt == 0 else (8 * tt + 2, 2)
                if spill is not None and kb < sp_dl:
                    if not drain(spill, sp_rate):
                        spill = None

                if kb + 2 < nkb:
                    emit_score(kb + 2)
                s = s_tiles[kb]
                c0 = _c0_of(tt, kb)
                if kb >= 8 * tt:
                    j = kb - 8 * tt
                    nc.vector.tensor_add(
                        s[:, c0:c0 + P], s[:, c0:c0 + P],
                        mask_sb[:, j, c0:c0 + P])
                pt = pt_pool.tile([P, 512], f16, tag="pt")
                nc.scalar.activation(pt[:, c0:512], s[:, c0:512], EXP,
                                     scale=SCALE)
                nc.tensor.matmul(
                    ot[:, c0:512],
                    lhsT=v_all[:, kb, :],
                    rhs=pt[:, c0:512],
                    start=(kb == 0),
                    stop=(kb == nkb - 1),
                )
                nc.vector.tensor_add(pacc[kb % 2][:, c0:512],
                                     pacc[kb % 2][:, c0:512],
                                     pt[:, c0:512])

                if gen_head is not None:
                    if not drain(gen_head, per_head):
                        gen_head = None
                elif gen_tail is not None and kb >= nkb - 8:
                    if not drain(gen_tail, 3):
                        gen_tail = None

            # anything left of the head must land now; tail becomes spill
            while drain(gen_head, 4):
                pass
            gen_head = None
            spill = gen_tail
            gen_tail = None

            # epilogue: softmax denominator + normalize + store (the last
            # tile streams in halves so the store overlaps the arithmetic)
            halves = (0, 256) if tt == N_RND - 1 else (0,)
            width = 512 // len(halves)
            for hb in halves:
                sl = slice(hb, hb + width)
                nc.vector.tensor_add(pacc[0][:, sl], pacc[0][:, sl],
                                     pacc[1][:, sl])
                nc.tensor.matmul(lb[:, sl], lhsT=allones[:],
                                 rhs=pacc[0][:, sl], start=True, stop=True)
                rl = rl_pool.tile([P, width], f32, tag="rl")
                nc.vector.reciprocal_approx_fast(rl[:], lb[:, sl])
                o_sb = osb_pool.tile([P, width], f32, tag="osb")
                nc.vector.tensor_mul(o_sb[:], ot[:, sl], rl[:])
                nc.gpsimd.dma_start(
                    out[:, tt * 512 + hb: tt * 512 + hb + width], o_sb[:])


def build_program():
    import concourse.tile as tile
    from concourse import bacc, mybir

    f32 = mybir.dt.float32
    f16 = mybir.dt.float16
    nc = bacc.Bacc("TRN2", target_bir_lowering=False, debug=False,
                   num_devices=N_CORES)
    xT = nc.dram_tensor("xT", [E, T], f16, kind="ExternalInput").ap()
    wqkv = nc.dram_tensor("wqkv", [E, 3, H], f16, kind="ExternalInput").ap()
    maskT = nc.dram_tensor("maskT", [1024, 512], f16, kind="ExternalInput").ap()
    out = nc.dram_tensor("out", [H, TQ], f32, kind="ExternalOutput").ap()

    with tile.TileContext(nc) as tc:
        _emit(tc, (xT, wqkv, maskT, out))
    nc.compile()
    return nc


def make_in_maps(x, Wq, Wk, Wv):
    """Per-core input maps. x: [B,T,E] f32; W*: [H,E] f32."""
    x = np.asarray(x, dtype=F32)
    # combined [E, 3, H] with slot order (k, v, q)
    wqkv = np.stack(
        [np.asarray(Wk, F32).T, np.asarray(Wv, F32).T, np.asarray(Wq, F32).T],
        axis=1).astype(np.float16)
    wqkv = np.ascontiguousarray(wqkv)
    masks = [_mask_strip(0), _mask_strip(1)]
    perms = [_perm_cols(0), _perm_cols(1)]
    in_maps = []
    for c in range(N_CORES):
        b, p = c // 2, c % 2
        xb = x[b][perms[p]]                                    # [T, E] permuted
        xT_np = np.ascontiguousarray(xb.T.astype(np.float16))
        in_maps.append({
            "xT": xT_np,
            "wqkv": wqkv,
            "maskT": masks[p],
        })
    return in_maps


def run(x, Wq, Wk, Wv, trace=False, trace_cores=None):
    """Returns (full_output [B,T,H] f32, BassKernelResults)."""
    from concourse.bass_utils import run_bass_kernel_spmd

    nc = build_program()
    in_maps = make_in_maps(x, Wq, Wk, Wv)
    res = run_bass_kernel_spmd(
        nc, in_maps, list(range(N_CORES)), trace=trace,
        trace_cores=trace_cores,
    )
    full = np.empty((B, T, H), dtype=F32)
    for c in range(N_CORES):
        b, p = c // 2, c % 2
        full[b, _query_rows(p), :] = res.results[c]["out"].T
    return full, res


def kernel(x, Wq, Wk, Wv):
    full, _ = run(x, Wq, Wk, Wv, trace=False)
    return full


if __name__ == "__main__":
    nc = build_program()
    print("program built ok")


# revision 37
# speedup vs baseline: 1.0068x; 1.0007x over previous
"""Single-head causal attention (B=4, T=4096, E=1024, H=128) on 8 trn2 cores.

Sharding: core c -> (batch b = c//2, piece p = c%2). Within a batch the 32
query blocks of 128 rows are split even/odd between the two pieces so the
causal workload balances. The device program is identical on all cores
(SPMD); all per-core differences are carried by the input data:
  - xT arrives column-PERMUTED per core: within every 256-token block the
    core's own 128 query tokens come first. The Q projection then reads a
    fixed strided slice of the same x tiles used for K/V (no separate
    gathered copy), and the program stays core-independent.
  - the causal-boundary mask strip is per-core data.

Device algorithm (per core, all "transposed" layouts):
  per round tt (1024 permuted tokens = key blocks 8tt..8tt+7):
    KT chunk = Wk @ x^T            [H=128, 512] x2
    QT tile  = Wq @ xq^T           [H=128, 512]   (strided in-tile gather)
    VT chunk = Wv @ x^T -> f16 -> PE-transpose -> V blocks [128 tok, 128 h]
    attention for q-tile tt over kb in 0..8tt+7:
      ST[kb] = KT_blk^T @ QT_tile  [128 k, 512 q] (PSUM, c0-trimmed)
      boundary block gets an additive causal mask strip (per-core data)
      PT = exp(scale*ST) f16; OT += V_blk^T @ PT; pacc[kb%2] += PT (DVE)
      lb = allones^T @ (pacc0+pacc1)   [128, 512] broadcast denominator
      O = OT * reciprocal_approx(lb) stored as out[:, tile] in [H, TQ]
  projections are software-pipelined: Q/K-A/V-A of round tt+1 interleave
  into attention round tt; K-B/V-B may spill into attention round tt+1
  itself (their key blocks are only read from iteration 8(tt+1)+4).
Host transposes [H, TQ] -> [TQ, H] when scattering into the full output.
"""

import numpy as np

B, T, E, H = 4, 4096, 1024, 128
P = 128
NB_E = E // P           # 8 contraction chunks
TQ = T // 2             # 2048 gathered queries per core
N_RND = 4               # rounds; round tt = permuted tokens 1024tt..+1023
SCALE = float(H) ** -0.5
NEG = -30000.0
N_CORES = 8
F32 = np.float32


def _query_rows(p: int) -> np.ndarray:
    """Absolute row indices of the gathered queries for piece p (in order)."""
    blocks = [np.arange(256 * g + 128 * p, 256 * g + 128 * p + 128) for g in range(16)]
    return np.concatenate(blocks)


def _perm_cols(p: int) -> np.ndarray:
    """Permuted token order for core piece p: within each 256-token block the
    own 128 tokens (offset 128p) come first, the other 128 after."""
    out = []
    for g in range(16):
        own = np.arange(256 * g + 128 * p, 256 * g + 128 * p + 128)
        oth = np.arange(256 * g + 128 * (1 - p), 256 * g + 128 * (1 - p) + 128)
        out.append(own)
        out.append(oth)
    return np.concatenate(out)


def _mask_strip(p: int) -> np.ndarray:
    """maskT [1024 k, 512 q] f16: 0 where key visible, NEG where masked.

    Row 128*j + kk is PERMUTED in-strip key block j (j=0..7); col 128*i + r
    is in-tile query block i. Permuted block j holds original block
    jp = j + p*(1-2*(j%2)); visible iff 128*jp + kk <= 256*i + 128*p + r.
    """
    j = np.arange(1024)[:, None] // 128
    kk = np.arange(1024)[:, None] % 128
    jp = j + p * (1 - 2 * (j % 2))
    qq = np.arange(512)[None, :]
    i, r = qq // 128, qq % 128
    visible = 128 * jp + kk <= 256 * i + 128 * p + r
    return np.where(visible, 0.0, NEG).astype(np.float16)


def _c0_of(tt: int, kb: int) -> int:
    """First un-skippable query column for key block kb in q-tile tt."""
    if kb < 8 * tt:
        return 0
    j = kb - 8 * tt
    return P * max(0, -(-(128 * j - 255) // 256))


def _emit(tc, aps):
    from concourse import mybir

    nc = tc.nc
    f32 = mybir.dt.float32
    f16 = mybir.dt.float16
    f8 = mybir.dt.float8e4
    DR = mybir.MatmulPerfMode.DoubleRow
    EXP = mybir.ActivationFunctionType.Exp

    from concourse.masks import make_identity
    from contextlib import ExitStack

    xT, wqkv, maskT, out = aps

    ctx = ExitStack()
    with ctx:
        # ---- pools ----
        consts = ctx.enter_context(tc.tile_pool(name="consts", bufs=1))
        x_pool = ctx.enter_context(tc.tile_pool(name="x", bufs=3))
        qt_pool = ctx.enter_context(tc.tile_pool(name="qt", bufs=2))
        vt_pool = ctx.enter_context(tc.tile_pool(name="vt", bufs=2))
        pt_pool = ctx.enter_context(tc.tile_pool(name="pt", bufs=4))
        pa_pool = ctx.enter_context(tc.tile_pool(name="pa", bufs=4))
        osb_pool = ctx.enter_context(tc.tile_pool(name="osb", bufs=2))
        rl_pool = ctx.enter_context(tc.tile_pool(name="rl", bufs=2))
        s_ps = ctx.enter_context(tc.tile_pool(name="sps", bufs=3, space="PSUM"))
        o_ps = ctx.enter_context(tc.tile_pool(name="ops", bufs=2, space="PSUM"))
        p_ps = ctx.enter_context(tc.tile_pool(name="pps", bufs=2, space="PSUM"))
        lb_ps = ctx.enter_context(tc.tile_pool(name="lps", bufs=1, space="PSUM"))

        # ---- persistent SBUF tensors ----
        identity = consts.tile([P, P], f16)
        allones = consts.tile([P, P], f16)
        w_sb = consts.tile([P, NB_E, 3, P], f16)   # [., chunk, (k|v|q), .]
        mask_sb = consts.tile([P, 8, 512], f16)
        kt_all = consts.tile([P, T // P, P], f16)
        v_all = consts.tile([P, T // P, P], f16)

        make_identity(nc, identity[:])
        nc.gpsimd.memset(allones[:], 1.0)

        # ---- x round tiles: [128, chunk c, block b, 256] f16 ----
        x_tiles = [x_pool.tile([P, NB_E, 4, 256], f16, tag="x", name=f"x_{tt}")
                   for tt in range(N_RND)]

        def dma_x(tt, halves, split=1):
            """Load round tt's 1024 permuted tokens (halves: list of 0/1).
            split>1 slices the load along E-chunks for finer semaphores."""
            for h in halves:
                t0 = tt * 1024 + h * 512
                cw = NB_E // split
                for g in range(split):
                    nc.sync.dma_start(
                        x_tiles[tt][:, g * cw:(g + 1) * cw, 2 * h:2 * h + 2, :],
                        xT[g * cw * P:(g + 1) * cw * P, t0:t0 + 512].rearrange(
                            "(c p) (b q) -> p c b q", p=P, b=2),
                    )

        # startup-critical order: weights, then round-0 x at chunk-pair
        # granularity (the first K matmuls gate only on their own chunks),
        # mask strips after (first needed mid-attention-0), bulk x last.
        for ch in (0, 4):
            nc.sync.dma_start(
                w_sb[:, ch:ch + 4, :, :],
                wqkv[ch * P:(ch + 4) * P, :, :].rearrange(
                    "(c p) s h -> p c s h", p=P))
        dma_x(0, [0], split=4)
        dma_x(0, [1], split=2)
        for jh in (0, 1):
            nc.sync.dma_start(
                mask_sb[:, 4 * jh:4 * jh + 4, :],
                maskT[512 * jh:512 * jh + 512, :].rearrange(
                    "(j p) q -> p j q", p=P))
        dma_x(1, [0, 1])
        dma_x(2, [0, 1])

        # PE warmup: dummy transposes ramp the tensor-engine p-state while
        # the first DMAs stream; their results are never read.
        for _ in range(48):
            wp = p_ps.tile([P, P], f16, tag="pps", name="warm")
            nc.tensor.transpose(wp[:], identity[:], identity[:])

        # ---- projection pieces (generators of thunks) ----
        def mk_group(tt, sel, rhs_fn, fin):
            ps = p_ps.tile([P, 512], f32, tag="pps")
            for c in range(NB_E):
                def mm(c=c, ps=ps):
                    nc.tensor.matmul(ps[:], lhsT=w_sb[:, c, sel, :],
                                     rhs=rhs_fn(c), start=(c == 0),
                                     stop=(c == NB_E - 1))
                yield mm
            yield lambda ps=ps: fin(ps)

        def v_group(tt, h):
            xt = x_tiles[tt]
            vt = vt_pool.tile([P, 512], f16, tag="vt", name=f"vt_{tt}_{h}")

            def fin_v(ps, vt=vt):
                nc.scalar.copy(vt[:], ps[:])
            yield from mk_group(
                tt, 1, lambda c, h=h: xt[:, c, 2 * h:2 * h + 2, :], fin_v)
            for u in range(4):
                kb = tt * 8 + 4 * h + u

                def tr(u=u, kb=kb, vt=vt):
                    tp = p_ps.tile([P, P], f16, tag="pps",
                                   name=f"tp_{tt}_{kb}")
                    nc.tensor.transpose(tp[:], vt[:, u * P:(u + 1) * P],
                                        identity[:])
                    nc.vector.tensor_copy(v_all[:, kb, :], tp[:])
                yield tr

        def k_group(tt, h):
            xt = x_tiles[tt]

            def fin_k(ps):
                dst = kt_all[:, tt * 8 + 4 * h: tt * 8 + 4 * h + 4, :]
                nc.scalar.copy(dst, ps[:])
            yield from mk_group(
                tt, 0, lambda c: xt[:, c, 2 * h:2 * h + 2, :], fin_k)

        def q_group(tt, qt):
            xt = x_tiles[tt]

            def fin_q(ps):
                nc.scalar.copy(qt[:], ps[:])
            yield from mk_group(tt, 2, lambda c: xt[:, c, :, 0:128], fin_q)

        def chain(*gens):
            for g in gens:
                yield from g

        def drain(gen, n):
            """Emit up to n pieces; returns False when exhausted."""
            if gen is None:
                return False
            for _ in range(n):
                try:
                    next(gen)()
                except StopIteration:
                    return False
            return True

        qts = [qt_pool.tile([P, 512], f16, tag="qt", name=f"qt_{t}")
               for t in range(N_RND)]

        # round 0 projections run up front (halves as their DMAs land)
        for piece in chain(k_group(0, 0), v_group(0, 0), k_group(0, 1),
                           q_group(0, qts[0])):
            piece()

        # interleave/spill generators:
        #   head(r) = Q + K-A + V-A of round r  (into attention r-1)
        #   tail(r) = K-B + V-B of round r      (may spill into attention r)
        head = lambda r: chain(q_group(r, qts[r]), k_group(r, 0),
                               v_group(r, 0))
        tail = lambda r: chain(k_group(r, 1), v_group(r, 1))

        gen_head = None   # projections of round tt+1, must finish in attn tt
        gen_tail = None   # projections of round tt+1, may spill into attn tt+1
        spill = v_group(0, 1)  # leftovers with deadline in the current round

        for tt in range(N_RND):
            if tt == 1:
                dma_x(3, [0, 1])
            if tt + 1 < N_RND:
                gen_head = chain(head(tt + 1))
                gen_tail = chain(tail(tt + 1))
            else:
                gen_head = gen_tail = None

            nkb = 8 * tt + 8
            # pacing: head must drain within this round's attention;
            # tail may run behind and spill (deadline in next round).
            n_head = 31  # 3 groups x 9 + 4 transposes
            per_head = -(-n_head // max(1, nkb - 2))

            qs = qts[tt]
            ot = o_ps.tile([P, 512], f32, tag="ops", name=f"ot_{tt}")
            lb = lb_ps.tile([P, 512], f32, tag="lps", name=f"lb_{tt}")
            pacc = [pa_pool.tile([P, 512], f16, tag="pa", name=f"pa_{tt}_{i}")
                    for i in range(2)]
            for i in range(2):
                nc.gpsimd.memset(pacc[i][:], 0.0)

            s_tiles = [None] * nkb

            def emit_score(kb, tt=tt, qs=qs, s_tiles=s_tiles):
                c0 = _c0_of(tt, kb)
                s = s_ps.tile([P, 512], f32, tag="sps", name=f"s_{tt}_{kb}")
                nc.tensor.matmul(
                    s[:, c0:512],
                    lhsT=kt_all[:, kb, :],
                    rhs=qs[:, c0:512],
                    start=True, stop=True,
                )
                s_tiles[kb] = s

            emit_score(0)
            if nkb > 1:
                emit_score(1)
            for kb in range(nkb):
                # spilled tail of THIS round's projections: its key blocks
                # (8tt+4..8tt+7) are first read at iteration 8tt+3 (score
                # lookahead); keep a safe margin.
                sp_dl, sp_rate = (4, 4) if tt == 0 else (8 * tt + 2, 2)
                if spill is not None and kb < sp_dl:
                    if not drain(spill, sp_rate):
                        spill = None

                if kb + 2 < nkb:
                    emit_score(kb + 2)
                s = s_tiles[kb]
                c0 = _c0_of(tt, kb)
                if kb >= 8 * tt:
                    j = kb - 8 * tt
                    nc.vector.tensor_add(
                        s[:, c0:c0 + P], s[:, c0:c0 + P],
                        mask_sb[:, j, c0:c0 + P])
                pt = pt_pool.tile([P, 512], f16, tag="pt")
                nc.scalar.activation(pt[:, c0:512], s[:, c0:512], EXP,
                                     scale=SCALE)
                nc.tensor.matmul(
                    ot[:, c0:512],
                    lhsT=v_all[:, kb, :],
                    rhs=pt[:, c0:512],
                    start=(kb == 0),
                    stop=(kb == nkb - 1),
                )
                nc.vector.tensor_add(pacc[kb % 2][:, c0:512],
                                     pacc[kb % 2][:, c0:512],
                                     pt[:, c0:512])

                if gen_head is not None:
                    if not drain(gen_head, per_head):
                        gen_head = None
                elif gen_tail is not None and kb >= nkb - 8:
                    if not drain(gen_tail, 3):
                        gen_tail = None

            # anything left of the head must land now; tail becomes spill
            while drain(gen_head, 4):
                pass
            gen_head = None
            spill = gen_tail
            gen_tail = None

            # epilogue: softmax denominator + normalize + store (the last
            # tile streams in halves so the store overlaps the arithmetic)
            halves = (0, 256) if tt == N_RND - 1 else (0,)
            width = 512 // len(halves)
            for hb in halves:
                sl = slice(hb, hb + width)
                nc.tensor.matmul(lb[:, sl], lhsT=allones[:],
                                 rhs=pacc[0][:, sl], start=True, stop=False)
                nc.tensor.matmul(lb[:, sl], lhsT=allones[:],
                                 rhs=pacc[1][:, sl], start=False, stop=True)
                rl = rl_pool.tile([P, width], f32, tag="rl")
                nc.vector.reciprocal_approx_fast(rl[:], lb[:, sl])
                o_sb = osb_pool.tile([P, width], f32, tag="osb")
                nc.vector.tensor_mul(o_sb[:], ot[:, sl], rl[:])
                nc.gpsimd.dma_start(
                    out[:, tt * 512 + hb: tt * 512 + hb + width], o_sb[:])


def build_program():
    import concourse.tile as tile
    from concourse import bacc, mybir

    f32 = mybir.dt.float32
    f16 = mybir.dt.float16
    nc = bacc.Bacc("TRN2", target_bir_lowering=False, debug=False,
                   num_devices=N_CORES)
    xT = nc.dram_tensor("xT", [E, T], f16, kind="ExternalInput").ap()
    wqkv = nc.dram_tensor("wqkv", [E, 3, H], f16, kind="ExternalInput").ap()
    maskT = nc.dram_tensor("maskT", [1024, 512], f16, kind="ExternalInput").ap()
    out = nc.dram_tensor("out", [H, TQ], f32, kind="ExternalOutput").ap()

    with tile.TileContext(nc) as tc:
        _emit(tc, (xT, wqkv, maskT, out))
    nc.compile()
    return nc


def make_in_maps(x, Wq, Wk, Wv):
    """Per-core input maps. x: [B,T,E] f32; W*: [H,E] f32."""
    x = np.asarray(x, dtype=F32)
    # combined [E, 3, H] with slot order (k, v, q)
    wqkv = np.stack(
        [np.asarray(Wk, F32).T, np.asarray(Wv, F32).T, np.asarray(Wq, F32).T],
        axis=1).astype(np.float16)
    wqkv = np.ascontiguousarray(wqkv)
    masks = [_mask_strip(0), _mask_strip(1)]
    perms = [_perm_cols(0), _perm_cols(1)]
    in_maps = []
    for c in range(N_CORES):
        b, p = c // 2, c % 2
        xb = x[b][perms[p]]                                    # [T, E] permuted
        xT_np = np.ascontiguousarray(xb.T.astype(np.float16))
        in_maps.append({
            "xT": xT_np,
            "wqkv": wqkv,
            "maskT": masks[p],
        })
    return in_maps


def run(x, Wq, Wk, Wv, trace=False, trace_cores=None):
    """Returns (full_output [B,T,H] f32, BassKernelResults)."""
    from concourse.bass_utils import run_bass_kernel_spmd

    nc = build_program()
    in_maps = make_in_maps(x, Wq, Wk, Wv)
    res = run_bass_kernel_spmd(
        nc, in_maps, list(range(N_CORES)), trace=trace,
        trace_cores=trace_cores,
    )
    full = np.empty((B, T, H), dtype=F32)
    for c in range(N_CORES):
        b, p = c // 2, c % 2
        full[b, _query_rows(p), :] = res.results[c]["out"].T
    return full, res


def kernel(x, Wq, Wk, Wv):
    full, _ = run(x, Wq, Wk, Wv, trace=False)
    return full


if __name__ == "__main__":
    nc = build_program()
    print("program built ok")
